# revision 21
# baseline (speedup 1.0000x reference)
"""Trainium2 Bass kernel for nn_MultiHeadAttention_81149112090633.

Math (faithful to the quirky reference):
  energy[q,k,n,h] = sum_d query[n,h,q*64+d] * keys[n,h,k*64+d]
  energy masked with -inf where mask[n,h]==0, softmax over the BATCH axis n,
  out[q,k,n,d] = sum_h att[q,k,n,h] * vsum[n,h,d],  vsum = sum_vh values[n,h,vh*64+d]
  final = rows(k,n) x features(q,d) matrix,  Y = X @ w_out.T + b_out

Sharding: data-parallel over batch n (32 per core x 8 cores). The softmax
couples cores only through the per-(q,k,h) denominator S = sum_n exp(...);
S is combined with an on-device AllReduce (1 MB), everything else is local.

Per-core phases:
  P1: per n: 64 tiny matmuls -> energy psum [128,2048] (partition=(h%2)*64+q,
      col=(h//2)*64+k); masking folded into the matmul via an augmented 65th
      contraction row. ACT exp (scale=1/8) -> expm bf16; S += expm (split
      DVE/Pool, fp32); expm spilled to HBM. (V loads deferred to phase 2.)
  CC: AllReduce S; transposed readback -> S^T [h, (q,k)]; reciprocal -> RSb.
  P2: per n: transposed readback of expm -> att^T [h, (q,k)]; multiply by
      RSb; einsum2 matmuls put d on psum partitions par*64 (par=q%2) so the
      psum layout matches X^T directly; convert psum -> X^T as SPLIT fp8:
      hi = fp8(x) (ACT), lo = fp8(x - hi) (DVE). alpha=2 is folded into
      VSALL so psum already holds alpha*X.
  P3: Y = X @ W^T as fp8 DoubleRow matmuls (2 k-tiles of 128 per instr,
      0.5 cycles/row): per 256-wide output chunk accumulate 16 ci-pairs x
      3 terms (Xh*Wh + Xh*Wl + Xl*Wh); W is pre-split hi/lo fp8 on the host
      scaled by beta=64. Epilogue: ACT copy psum*(1/(alpha*beta)) -> bf16,
      Pool adds bias, DMA out (bf16; host converts to f32).
      P3 is interleaved with P2 in a staircase so the PE never starves; W is
      streamed twice total.

The h (seq) axis is stored partition-permuted (evens then odds) in phase 2;
values rows are pre-permuted on the host to match.
"""

import os

import numpy as np
import ml_dtypes

N, L, H, D, E = 256, 64, 64, 64, 4096
NCORES = 8
NN = N // NCORES  # 32 batch elements per core
NEG = -2000.0  # mask bias pre exp-scale (exp((e-2000)/8) == 0 in fp32)
ALPHA = 2.0  # X scale into fp8 (folded into VSALL)
BETA = 64.0  # W scale into fp8 (folded into host W prep)

# partition p in phase-2 h-layout corresponds to seq position PERM[p]
PERM = np.array([2 * p for p in range(32)] + [2 * p + 1 for p in range(32)])

_PROGRAM_CACHE = {}


def build_program(nn=NN, n_cores=NCORES, use_collective=True, max_phase=3):
    """Build the Bass/Tile SPMD program (one NeuronCore's instruction stream)."""
    import concourse.bass as bass
    import concourse.mybir as mybir
    import concourse.tile as tile
    from concourse import bacc

    f32 = mybir.dt.float32
    bf16 = mybir.dt.bfloat16
    f8 = mybir.dt.float8e4
    AF = mybir.ActivationFunctionType
    ALU = mybir.AluOpType
    DR = mybir.MatmulPerfMode.DoubleRow
    R = nn * 64  # output rows per core

    nc = bacc.Bacc(trn_type="TRN2", num_devices=n_cores)

    QKT = nc.dram_tensor("qkt", [nn, 65, 2 * E], bf16, kind="ExternalInput").ap()
    VT = nc.dram_tensor("vt", [nn, L, E], bf16, kind="ExternalInput").ap()
    WH8 = nc.dram_tensor("wh8", [128, 16 * 32 * 256], f8, kind="ExternalInput").ap()
    WL8 = nc.dram_tensor("wl8", [128, 16 * 32 * 256], f8, kind="ExternalInput").ap()
    BB = nc.dram_tensor("bb", [128, E], bf16, kind="ExternalInput").ap()
    OUT = nc.dram_tensor("out", [R, E], bf16, kind="ExternalOutput").ap()
    EXPM = nc.dram_tensor("expmbuf", [nn, 128, 2048], bf16, kind="Internal").ap()
    CCIN = nc.dram_tensor("ccin", [128, 2048], f32, kind="Internal").ap()
    CCOUT = nc.dram_tensor(
        "ccout", [128, 2048], f32, kind="Internal", addr_space="Shared"
    ).ap()

    with tile.TileContext(nc) as tc:
        with tc.tile_pool(name="persist", bufs=1) as persist:
            VSALL = persist.tile([64, 64 * nn], bf16, tag="VSALL")
            RSb = persist.tile([64, E], bf16, tag="RSb")

            # ---------------- Phase 1 ----------------
            with (
                tc.tile_pool(name="p1", bufs=4) as p1,
                tc.tile_pool(name="p1q", bufs=5) as p1q,
                tc.tile_pool(name="p1s", bufs=1) as p1s,
                tc.tile_pool(name="ps1", bufs=2, space="PSUM") as psp,
            ):
                S = p1s.tile([128, 2048], f32, tag="S")
                for nl in range(nn):
                    qka = p1q.tile([65, 2 * E], bf16, tag="qka")
                    nc.sync.dma_start(qka[:], QKT[nl])
                    ps = psp.tile([128, 2048], f32, tag="ps")
                    for h in range(L):
                        par, j = h % 2, h // 2
                        nc.tensor.matmul(
                            ps[par * 64 : par * 64 + 64, j * 64 : j * 64 + 64],
                            qka[:, h * 64 : h * 64 + 64],
                            qka[:, E + h * 64 : E + h * 64 + 64],
                            start=True,
                            stop=True,
                        )
                    expm = p1.tile([128, 2048], bf16, tag="expm")
                    nc.scalar.activation(expm[:], ps[:], AF.Exp, scale=0.125)
                    if nl == 0:
                        nc.vector.tensor_copy(S[:], expm[:])
                    else:
                        nc.vector.tensor_add(S[:], S[:], expm[:])
                    nc.sync.dma_start(EXPM[nl], expm[:])

                # ---------------- AllReduce of S ----------------
                # Split into two q-halves pipelined so phase 2 can start on
                # the first half while the second is still in flight.
                ccr = CCOUT.rearrange("(par q) (j k) -> par j q k", par=2, k=64)
                ST = p1s.tile([64, E], f32, tag="ST")
                for qh in range(2):
                    r0 = qh * 32
                    nc.sync.dma_start(CCIN[r0 : r0 + 32], S[r0 : r0 + 32])
                    nc.sync.dma_start(CCIN[64 + r0 : 96 + r0], S[64 + r0 : 96 + r0])
                    if use_collective:
                        nc.gpsimd.collective_compute(
                            "AllReduce",
                            mybir.AluOpType.add,
                            replica_groups=[list(range(n_cores))],
                            ins=[CCIN[r0 : r0 + 32]],
                            outs=[CCOUT[r0 : r0 + 32]],
                        )
                        nc.gpsimd.collective_compute(
                            "AllReduce",
                            mybir.AluOpType.add,
                            replica_groups=[list(range(n_cores))],
                            ins=[CCIN[64 + r0 : 96 + r0]],
                            outs=[CCOUT[64 + r0 : 96 + r0]],
                        )
                    else:
                        nc.sync.dma_start(CCOUT[r0 : r0 + 32], CCIN[r0 : r0 + 32])
                        nc.sync.dma_start(
                            CCOUT[64 + r0 : 96 + r0], CCIN[64 + r0 : 96 + r0]
                        )
                    # transposed readback: S^T[p, q*64+k] with h=PERM[p]
                    nc.sync.dma_start(
                        ST[0:32, qh * 2048 : qh * 2048 + 2048], ccr[0][:, r0 : r0 + 32]
                    )
                    nc.sync.dma_start(
                        ST[32:64, qh * 2048 : qh * 2048 + 2048], ccr[1][:, r0 : r0 + 32]
                    )
                    with nc.allow_low_precision(reason="recip feeds bf16 att anyway"):
                        nc.vector.reciprocal(
                            RSb[:, qh * 2048 : qh * 2048 + 2048],
                            ST[:, qh * 2048 : qh * 2048 + 2048],
                        )

            # ---------------- Phase 2 + 3 ----------------
            with (
                tc.tile_pool(name="xt", bufs=1) as xtp,
                tc.tile_pool(name="p2", bufs=3) as p2,
                tc.tile_pool(name="pvt", bufs=1) as pvt,
                tc.tile_pool(name="p3w", bufs=2) as p3w,
                tc.tile_pool(name="p3b", bufs=2) as p3b,
                tc.tile_pool(name="p3y", bufs=2) as p3y,
                tc.tile_pool(name="ps2p", bufs=3, space="PSUM") as ps2p,
                tc.tile_pool(name="psyp", bufs=3, space="PSUM") as psyp,
            ):
                XTH = xtp.tile([128, 32 * R], f8, tag="XTH")
                XTL = xtp.tile([128, 32 * R], f8, tag="XTL")
                XTHv = XTH.rearrange("p (ci r) -> p ci r", r=R)
                XTLv = XTL.rearrange("p (ci r) -> p ci r", r=R)

                def load_vt(nl, q=None):
                    vt = pvt.tile([L, E], bf16, tag="vt")
                    (q or nc.scalar).dma_start(vt[:], VT[nl])
                    vsb = p2.tile([64, 64], bf16, tag="vsb")
                    with nc.allow_low_precision(reason="vsum is stored bf16 anyway"):
                        nc.vector.reduce_sum(
                            vsb[:],
                            vt.rearrange("p (d vh) -> p d vh", vh=64),
                            axis=mybir.AxisListType.X,
                        )
                    nc.scalar.activation(
                        VSALL[:, nl * 64 : nl * 64 + 64], vsb[:], AF.Copy, scale=ALPHA
                    )

                araws = {}

                def prefetch_araw(nl):
                    araw = p2.tile([64, E], bf16, tag="araw")
                    er = EXPM[nl].rearrange("(par q) (j k) -> par j q k", par=2, k=64)
                    nc.sync.dma_start(araw[0:32, :], er[0])
                    nc.sync.dma_start(araw[32:64, :], er[1])
                    araws[nl] = araw

                wtiles = {}

                def load_w(oc):
                    wh = p3w.tile([128, 32 * 256], f8, tag="wh")
                    wl = p3w.tile([128, 32 * 256], f8, tag="wl")
                    nc.sync.dma_start(wh[:], WH8[:, oc * 8192 : oc * 8192 + 8192])
                    nc.sync.dma_start(wl[:], WL8[:, oc * 8192 : oc * 8192 + 8192])
                    bbt = p3b.tile([128, 256], bf16, tag="bbt")
                    nc.sync.dma_start(bbt[:], BB[:, oc * 256 : oc * 256 + 256])
                    wtiles[oc] = (wh, wl, bbt)

                def run_p3(oc, rc_list):
                    wh, wl, bbt = wtiles.pop(oc)
                    whv = wh.rearrange("p (ci o) -> p ci o", ci=32)
                    wlv = wl.rearrange("p (ci o) -> p ci o", ci=32)
                    for rc in rc_list:
                        psY = psyp.tile([128, 512], f32, tag="psy")
                        idx = 0
                        for j in range(16):
                            for xv, wv in ((XTHv, whv), (XTHv, wlv), (XTLv, whv)):
                                nc.tensor.matmul(
                                    psY[:, 0:256],
                                    xv[:, 2 * j : 2 * j + 2, rc * 128 : rc * 128 + 128],
                                    wv[:, 2 * j : 2 * j + 2, :],
                                    start=(idx == 0),
                                    stop=(idx == 47),
                                    perf_mode=DR,
                                )
                                idx += 1
                        t = p3y.tile([128, 256], bf16, tag="t")
                        nc.scalar.activation(
                            t[:], psY[:, 0:256], AF.Copy, scale=1.0 / (ALPHA * BETA)
                        )
                        yb = p3y.tile([128, 256], bf16, tag="yb")
                        nc.gpsimd.tensor_tensor(
                            out=yb[:], in0=t[:], in1=bbt[:], op=ALU.add
                        )
                        nc.gpsimd.dma_start(
                            OUT[rc * 128 : rc * 128 + 128, oc * 256 : oc * 256 + 256],
                            yb[:],
                        )

                # prefetches that overlap the S-collective chain
                load_vt(0)
                load_vt(1)
                prefetch_araw(0)
                prefetch_araw(1)
                load_w(0)

                for nl in range(nn if max_phase >= 2 else 0):
                    if nl + 2 < nn:
                        prefetch_araw(nl + 2)
                        load_vt(nl + 2)
                    araw = araws.pop(nl)
                    nc.vector.tensor_mul(
                        araw[:, 0:2048], araw[:, 0:2048], RSb[:, 0:2048]
                    )
                    nc.vector.tensor_mul(
                        araw[:, 2048:4096], araw[:, 2048:4096], RSb[:, 2048:4096]
                    )
                    arv = araw.rearrange("p (q k) -> p q k", k=64)
                    for quarter in range(4):
                        ps2 = ps2p.tile([128, 512], f32, tag="ps2")
                        for par in range(2):
                            nc.tensor.matmul(
                                ps2[par * 64 : par * 64 + 64, :],
                                VSALL[:, nl * 64 : nl * 64 + 64],
                                arv[:, quarter * 16 + par : quarter * 16 + 16 : 2, :],
                                start=True,
                                stop=True,
                            )
                        ps2v = ps2.rearrange("p (t k) -> p t k", k=64)
                        dsth = XTHv[
                            :, quarter * 8 : quarter * 8 + 8, nl * 64 : nl * 64 + 64
                        ]
                        nc.scalar.copy(dsth, ps2v)
                        nc.vector.tensor_tensor(
                            out=XTLv[
                                :, quarter * 8 : quarter * 8 + 8, nl * 64 : nl * 64 + 64
                            ],
                            in0=ps2v,
                            in1=dsth,
                            op=ALU.subtract,
                        )
                    # staircase: at pair k, prefetch W(k+1) and run oc=k over
                    # the rc's that are ready (0..k)
                    if max_phase >= 3 and nl % 2 == 1:
                        k = nl // 2
                        if k + 1 < 16:
                            load_w(k + 1)
                        run_p3(k, list(range(0, k + 1)))

                # tail: each oc chunk re-streamed for its remaining rc range
                if max_phase >= 3:
                    load_w(0)
                    for oc in range(15):
                        if oc + 1 < 15:
                            load_w(oc + 1)
                        run_p3(oc, list(range(oc + 1, 16)))

    nc.compile()
    return nc


def prep_inputs(inputs, nn=NN, n_cores=NCORES):
    """Host-side shard + layout prep. Returns list of per-core input maps."""
    q = np.asarray(inputs["query"], dtype=np.float32)
    k = np.asarray(inputs["keys"], dtype=np.float32)
    v = np.asarray(inputs["values"], dtype=np.float32)
    m = np.asarray(inputs["mask"])
    w = np.asarray(inputs["w_out"], dtype=np.float32)
    b = np.asarray(inputs["b_out"], dtype=np.float32)

    f8 = ml_dtypes.float8_e4m3
    ws = np.ascontiguousarray(w.T) * BETA  # [i, o] = [ci*128+p, oc*256+o]
    WHh = ws.astype(f8)
    WLh = (ws - WHh.astype(np.float32)).astype(f8)
    # pre-tile to [p, oc, ci, o] so per-oc-chunk DMAs are fully contiguous
    WHh = np.ascontiguousarray(
        WHh.reshape(32, 128, 16, 256).transpose(1, 2, 0, 3).reshape(128, 16 * 32 * 256)
    )
    WLh = np.ascontiguousarray(
        WLh.reshape(32, 128, 16, 256).transpose(1, 2, 0, 3).reshape(128, 16 * 32 * 256)
    )
    BBh = np.ascontiguousarray(
        np.broadcast_to(b.astype(np.float32), (128, E))
    ).astype(ml_dtypes.bfloat16)

    maps = []
    for c in range(n_cores):
        ns = slice(c * nn, (c + 1) * nn)
        qr = q[ns].reshape(nn, L, H, D)  # [nl, h, qh, d]
        kr = k[ns].reshape(nn, L, H, D)
        QTh = np.empty((nn, 65, L, H), np.float32)
        QTh[:, :64] = qr.transpose(0, 3, 1, 2)  # [nl, d, h, qh]
        QTh[:, 64] = 1.0
        KTh = np.empty((nn, 65, L, H), np.float32)
        KTh[:, :64] = kr.transpose(0, 3, 1, 2)
        KTh[:, 64] = (m[ns].astype(np.float32) - 1.0)[:, :, None] * (-NEG)
        vperm = v[ns][:, PERM, :].reshape(nn, L, H, D)  # [nl, l, vh, d]
        VTh = np.ascontiguousarray(
            vperm.transpose(0, 1, 3, 2).reshape(nn, L, E)
        ).astype(ml_dtypes.bfloat16)
        QKh = np.concatenate(
            [QTh.reshape(nn, 65, E), KTh.reshape(nn, 65, E)], axis=2
        ).astype(ml_dtypes.bfloat16)
        maps.append({"qkt": QKh, "vt": VTh, "wh8": WHh, "wl8": WLh, "bb": BBh})
    return maps


def assemble_output(core_outs, nn=NN, n_cores=NCORES):
    """core_outs[c] = [nn*64, E] with row nl*64+kh -> full (256, 64, E)."""
    n_total = nn * n_cores
    full = np.empty((H, n_total, E), np.float32)  # [kh, n]
    for c in range(n_cores):
        full[:, c * nn : (c + 1) * nn, :] = (
            np.asarray(core_outs[c]).astype(np.float32)
            .reshape(nn, H, E)
            .transpose(1, 0, 2)
        )
    return full.reshape(n_total, L, E)


def kernel(**inputs) -> np.ndarray:
    from concourse import bass_utils

    key = (NN, NCORES)
    if key not in _PROGRAM_CACHE:
        _PROGRAM_CACHE[key] = build_program(NN, NCORES)
    nc = _PROGRAM_CACHE[key]

    in_maps = prep_inputs(inputs, NN, NCORES)
    trace = bool(int(os.environ.get("KERNEL_TRACE", "0")))
    res = bass_utils.run_bass_kernel_spmd(
        nc,
        in_maps,
        core_ids=list(range(NCORES)),
        trace=trace,
        trace_cores=list(range(NCORES)) if trace else None,
    )
    if trace and res.exec_time_ns is not None:
        print(f"HW exec time: {res.exec_time_ns} ns")
        print(f"HW exec time mean: {res.mean_exec_time_ns} ns")
    core_outs = [r["out"] for r in res.results]
    return assemble_output(core_outs, NN, NCORES)


# revision 35
# speedup vs baseline: 1.0370x; 1.0370x over previous
"""Trainium2 Bass kernel for nn_MultiHeadAttention_81149112090633.

Math (faithful to the quirky reference):
  energy[q,k,n,h] = sum_d query[n,h,q*64+d] * keys[n,h,k*64+d]
  energy masked with -inf where mask[n,h]==0, softmax over the BATCH axis n,
  out[q,k,n,d] = sum_h att[q,k,n,h] * vsum[n,h,d],  vsum = sum_vh values[n,h,vh*64+d]
  final = rows(k,n) x features(q,d) matrix,  Y = X @ w_out.T + b_out

Sharding: data-parallel over batch n (32 per core x 8 cores). The softmax
couples cores only through the per-(q,k,h) denominator S = sum_n exp(...);
S is combined with an on-device AllReduce (1 MB), everything else is local.

Per-core phases:
  P1: per n: 64 tiny matmuls -> energy psum [128,2048] (partition=(h%2)*64+q,
      col=(h//2)*64+k); masking folded into the matmul via an augmented 65th
      contraction row. ACT exp (scale=1/8) -> expm bf16; S += expm (split
      DVE/Pool, fp32); expm spilled to HBM. (V loads deferred to phase 2.)
  CC: AllReduce S; transposed readback -> S^T [h, (q,k)]; reciprocal -> RSb.
  P2: per n: transposed readback of expm -> att^T [h, (q,k)]; multiply by
      RSb; einsum2 matmuls put d on psum partitions par*64 (par=q%2) so the
      psum layout matches X^T directly; convert psum -> X^T as SPLIT fp8:
      hi = fp8(x) (ACT), lo = fp8(x - hi) (DVE). alpha=2 is folded into
      VSALL so psum already holds alpha*X.
  P3: Y = X @ W^T as fp8 DoubleRow matmuls (2 k-tiles of 128 per instr,
      0.5 cycles/row): per 256-wide output chunk accumulate 16 ci-pairs x
      3 terms (Xh*Wh + Xh*Wl + Xl*Wh); W is pre-split hi/lo fp8 on the host
      scaled by beta=64. Epilogue: ACT copy psum*(1/(alpha*beta)) -> bf16,
      Pool adds bias, DMA out (bf16; host converts to f32).
      P3 is interleaved with P2 in a staircase so the PE never starves; W is
      streamed twice total.

The h (seq) axis is stored partition-permuted (evens then odds) in phase 2;
values rows are pre-permuted on the host to match.
"""

import os

import numpy as np
import ml_dtypes

N, L, H, D, E = 256, 64, 64, 64, 4096
NCORES = 8
NN = N // NCORES  # 32 batch elements per core
NEG = -2000.0  # mask bias pre exp-scale (exp((e-2000)/8) == 0 in fp32)
ALPHA = 2.0  # X scale into fp8 (folded into VSALL)
BETA = 64.0  # W scale into fp8 (folded into host W prep)

# partition p in phase-2 h-layout corresponds to seq position PERM[p]
PERM = np.array([2 * p for p in range(32)] + [2 * p + 1 for p in range(32)])

_PROGRAM_CACHE = {}


def build_program(nn=NN, n_cores=NCORES, use_collective=True, max_phase=3):
    """Build the Bass/Tile SPMD program (one NeuronCore's instruction stream)."""
    import concourse.bass as bass
    import concourse.mybir as mybir
    import concourse.tile as tile
    from concourse import bacc

    f32 = mybir.dt.float32
    bf16 = mybir.dt.bfloat16
    f8 = mybir.dt.float8e4
    AF = mybir.ActivationFunctionType
    ALU = mybir.AluOpType
    DR = mybir.MatmulPerfMode.DoubleRow
    R = nn * 64  # output rows per core

    nc = bacc.Bacc(trn_type="TRN2", num_devices=n_cores)

    QKT = nc.dram_tensor("qkt", [nn, 65, 2 * E], bf16, kind="ExternalInput").ap()
    VT = nc.dram_tensor("vt", [nn, L, E], bf16, kind="ExternalInput").ap()
    WH8 = nc.dram_tensor("wh8", [128, 16 * 32 * 256], f8, kind="ExternalInput").ap()
    WL8 = nc.dram_tensor("wl8", [128, 16 * 32 * 256], f8, kind="ExternalInput").ap()
    BB = nc.dram_tensor("bb", [128, E], bf16, kind="ExternalInput").ap()
    OUT = nc.dram_tensor("out", [R, E], bf16, kind="ExternalOutput").ap()
    f16 = mybir.dt.float16
    EXPM = nc.dram_tensor("expmbuf", [nn, 128, 2048], bf16, kind="Internal").ap()
    CCIN = nc.dram_tensor("ccin", [128, 2048], f16, kind="Internal").ap()
    CCOUT = nc.dram_tensor(
        "ccout", [128, 2048], f16, kind="Internal", addr_space="Shared"
    ).ap()

    with tile.TileContext(nc) as tc:
        with tc.tile_pool(name="persist", bufs=1) as persist:
            VSALL = persist.tile([64, 64 * nn], bf16, tag="VSALL")
            RSb = persist.tile([64, E], bf16, tag="RSb")

            # ---------------- Phase 1 ----------------
            with (
                tc.tile_pool(name="p1", bufs=6) as p1,
                tc.tile_pool(name="p1q", bufs=5) as p1q,
                tc.tile_pool(name="p1s", bufs=1) as p1s,
                tc.tile_pool(name="ps1", bufs=2, space="PSUM") as psp,
            ):
                S = p1s.tile([128, 2048], f16, tag="S")
                for nl in range(nn):
                    qka = p1q.tile([65, 2 * E], bf16, tag="qka")
                    nc.sync.dma_start(qka[:], QKT[nl])
                    ps = psp.tile([128, 2048], f32, tag="ps")
                    for h in range(L):
                        par, j = h % 2, h // 2
                        nc.tensor.matmul(
                            ps[par * 64 : par * 64 + 64, j * 64 : j * 64 + 64],
                            qka[:, h * 64 : h * 64 + 64],
                            qka[:, E + h * 64 : E + h * 64 + 64],
                            start=True,
                            stop=True,
                        )
                    expm = p1.tile([128, 2048], bf16, tag="expm")
                    nc.scalar.activation(expm[:], ps[:], AF.Exp, scale=0.125)
                    with nc.allow_low_precision(reason="fp16 softmax denom, ~0.1% err"):
                        if nl == 0:
                            nc.vector.tensor_copy(S[:], expm[:])
                        else:
                            nc.vector.tensor_add(S[:], S[:], expm[:])
                    nc.scalar.dma_start(EXPM[nl], expm[:])

                # ---------------- AllReduce of S ----------------
                # Split into two q-halves pipelined so phase 2 can start on
                # the first half while the second is still in flight.
                ccr = CCOUT.rearrange("(par q) (j k) -> par j q k", par=2, k=64)
                ST = p1s.tile([64, E], f16, tag="ST")
                for qh in range(2):
                    r0 = qh * 32
                    nc.sync.dma_start(CCIN[r0 : r0 + 32], S[r0 : r0 + 32])
                    nc.sync.dma_start(CCIN[64 + r0 : 96 + r0], S[64 + r0 : 96 + r0])
                    if use_collective:
                        nc.gpsimd.collective_compute(
                            "AllReduce",
                            mybir.AluOpType.add,
                            replica_groups=[list(range(n_cores))],
                            ins=[CCIN[r0 : r0 + 32]],
                            outs=[CCOUT[r0 : r0 + 32]],
                        )
                        nc.gpsimd.collective_compute(
                            "AllReduce",
                            mybir.AluOpType.add,
                            replica_groups=[list(range(n_cores))],
                            ins=[CCIN[64 + r0 : 96 + r0]],
                            outs=[CCOUT[64 + r0 : 96 + r0]],
                        )
                    else:
                        nc.sync.dma_start(CCOUT[r0 : r0 + 32], CCIN[r0 : r0 + 32])
                        nc.sync.dma_start(
                            CCOUT[64 + r0 : 96 + r0], CCIN[64 + r0 : 96 + r0]
                        )
                    # transposed readback: S^T[p, q*64+k] with h=PERM[p]
                    nc.sync.dma_start(
                        ST[0:32, qh * 2048 : qh * 2048 + 2048], ccr[0][:, r0 : r0 + 32]
                    )
                    nc.sync.dma_start(
                        ST[32:64, qh * 2048 : qh * 2048 + 2048], ccr[1][:, r0 : r0 + 32]
                    )
                    with nc.allow_low_precision(reason="recip feeds bf16 att anyway"):
                        nc.vector.reciprocal(
                            RSb[:, qh * 2048 : qh * 2048 + 2048],
                            ST[:, qh * 2048 : qh * 2048 + 2048],
                        )

            # ---------------- Phase 2 + 3 ----------------
            with (
                tc.tile_pool(name="xt", bufs=1) as xtp,
                tc.tile_pool(name="p2", bufs=3) as p2,
                tc.tile_pool(name="pvt", bufs=1) as pvt,
                tc.tile_pool(name="p3w", bufs=2) as p3w,
                tc.tile_pool(name="p3b", bufs=2) as p3b,
                tc.tile_pool(name="p3y", bufs=2) as p3y,
                tc.tile_pool(name="ps2p", bufs=3, space="PSUM") as ps2p,
                tc.tile_pool(name="psyp", bufs=4, space="PSUM") as psyp,
            ):
                XTH = xtp.tile([128, 32 * R], f8, tag="XTH")
                XTL = xtp.tile([128, 32 * R], f8, tag="XTL")
                XTHv = XTH.rearrange("p (ci r) -> p ci r", r=R)
                XTLv = XTL.rearrange("p (ci r) -> p ci r", r=R)

                def load_vt(nl, q=None):
                    vt = pvt.tile([L, E], bf16, tag="vt")
                    (q or nc.scalar).dma_start(vt[:], VT[nl])
                    vsb = p2.tile([64, 64], bf16, tag="vsb")
                    vtv = vt.rearrange("p (d vh) -> p d vh", vh=64)
                    with nc.allow_low_precision(reason="vsum is stored bf16 anyway"):
                        for dq in range(4):
                            nc.vector.reduce_sum(
                                vsb[:, dq * 16 : dq * 16 + 16],
                                vtv[:, dq * 16 : dq * 16 + 16, :],
                                axis=mybir.AxisListType.X,
                            )
                    nc.scalar.activation(
                        VSALL[:, nl * 64 : nl * 64 + 64], vsb[:], AF.Copy, scale=ALPHA
                    )

                araws = {}

                def prefetch_araw(nl):
                    araw = p2.tile([64, E], bf16, tag="araw")
                    er = EXPM[nl].rearrange("(par q) (j k) -> par j q k", par=2, k=64)
                    nc.sync.dma_start(araw[0:32, :], er[0])
                    nc.sync.dma_start(araw[32:64, :], er[1])
                    araws[nl] = araw

                wtiles = {}

                def load_w(oc):
                    wh = p3w.tile([128, 32 * 256], f8, tag="wh")
                    wl = p3w.tile([128, 32 * 256], f8, tag="wl")
                    nc.gpsimd.dma_start(wh[:], WH8[:, oc * 8192 : oc * 8192 + 8192])
                    nc.gpsimd.dma_start(wl[:], WL8[:, oc * 8192 : oc * 8192 + 8192])
                    bbt = p3b.tile([128, 256], bf16, tag="bbt")
                    nc.sync.dma_start(bbt[:], BB[:, oc * 256 : oc * 256 + 256])
                    wtiles[oc] = (wh, wl, bbt)

                def run_p3(oc, rc_list):
                    wh, wl, bbt = wtiles.pop(oc)
                    whv = wh.rearrange("p (ci o) -> p ci o", ci=32)
                    wlv = wl.rearrange("p (ci o) -> p ci o", ci=32)
                    for rc in rc_list:
                        psY = psyp.tile([128, 512], f32, tag="psy")
                        idx = 0
                        for j in range(16):
                            for xv, wv in ((XTHv, whv), (XTHv, wlv), (XTLv, whv)):
                                nc.tensor.matmul(
                                    psY[:, 0:256],
                                    xv[:, 2 * j : 2 * j + 2, rc * 128 : rc * 128 + 128],
                                    wv[:, 2 * j : 2 * j + 2, :],
                                    start=(idx == 0),
                                    stop=(idx == 47),
                                    perf_mode=DR,
                                )
                                idx += 1
                        t = p3y.tile([128, 256], bf16, tag="t")
                        nc.scalar.activation(
                            t[:], psY[:, 0:256], AF.Copy, scale=1.0 / (ALPHA * BETA)
                        )
                        yb = p3y.tile([128, 256], bf16, tag="yb")
                        nc.gpsimd.tensor_tensor(
                            out=yb[:], in0=t[:], in1=bbt[:], op=ALU.add
                        )
                        nc.gpsimd.dma_start(
                            OUT[rc * 128 : rc * 128 + 128, oc * 256 : oc * 256 + 256],
                            yb[:],
                        )

                # prefetches that overlap the S-collective chain
                prefetch_araw(0)
                prefetch_araw(1)
                load_vt(0, nc.sync)
                load_vt(1, nc.sync)
                load_w(0)

                for nl in range(nn if max_phase >= 2 else 0):
                    if max_phase >= 3 and nl % 2 == 0 and nl >= 2:
                        k = nl // 2
                        load_w(k)
                        run_p3(k - 1, list(range(0, k)))
                    if nl + 2 < nn:
                        prefetch_araw(nl + 2)
                        load_vt(nl + 2)
                    araw = araws.pop(nl)
                    nc.vector.tensor_mul(
                        araw[:, 0:2048], araw[:, 0:2048], RSb[:, 0:2048]
                    )
                    nc.vector.tensor_mul(
                        araw[:, 2048:4096], araw[:, 2048:4096], RSb[:, 2048:4096]
                    )
                    arv = araw.rearrange("p (q k) -> p q k", k=64)
                    for quarter in range(4):
                        ps2 = ps2p.tile([128, 512], f32, tag="ps2")
                        for par in range(2):
                            nc.tensor.matmul(
                                ps2[par * 64 : par * 64 + 64, :],
                                VSALL[:, nl * 64 : nl * 64 + 64],
                                arv[:, quarter * 16 + par : quarter * 16 + 16 : 2, :],
                                start=True,
                                stop=True,
                            )
                        ps2v = ps2.rearrange("p (t k) -> p t k", k=64)
                        dsth = XTHv[
                            :, quarter * 8 : quarter * 8 + 8, nl * 64 : nl * 64 + 64
                        ]
                        nc.scalar.copy(dsth, ps2v)
                        nc.vector.tensor_tensor(
                            out=XTLv[
                                :, quarter * 8 : quarter * 8 + 8, nl * 64 : nl * 64 + 64
                            ],
                            in0=ps2v,
                            in1=dsth,
                            op=ALU.subtract,
                        )
                # tail: oc 15 runs all rc, then each oc chunk is re-streamed
                # for its remaining rc range
                if max_phase >= 3:
                    run_p3(15, list(range(0, 16)))
                    load_w(0)
                    for oc in range(15):
                        if oc + 1 < 15:
                            load_w(oc + 1)
                        run_p3(oc, list(range(oc + 1, 16)))

    nc.compile()
    return nc


def prep_inputs(inputs, nn=NN, n_cores=NCORES):
    """Host-side shard + layout prep. Returns list of per-core input maps."""
    q = np.asarray(inputs["query"], dtype=np.float32)
    k = np.asarray(inputs["keys"], dtype=np.float32)
    v = np.asarray(inputs["values"], dtype=np.float32)
    m = np.asarray(inputs["mask"])
    w = np.asarray(inputs["w_out"], dtype=np.float32)
    b = np.asarray(inputs["b_out"], dtype=np.float32)

    f8 = ml_dtypes.float8_e4m3
    ws = np.ascontiguousarray(w.T) * BETA  # [i, o] = [ci*128+p, oc*256+o]
    WHh = ws.astype(f8)
    WLh = (ws - WHh.astype(np.float32)).astype(f8)
    # pre-tile to [p, oc, ci, o] so per-oc-chunk DMAs are fully contiguous
    WHh = np.ascontiguousarray(
        WHh.reshape(32, 128, 16, 256).transpose(1, 2, 0, 3).reshape(128, 16 * 32 * 256)
    )
    WLh = np.ascontiguousarray(
        WLh.reshape(32, 128, 16, 256).transpose(1, 2, 0, 3).reshape(128, 16 * 32 * 256)
    )
    BBh = np.ascontiguousarray(
        np.broadcast_to(b.astype(np.float32), (128, E))
    ).astype(ml_dtypes.bfloat16)

    maps = []
    for c in range(n_cores):
        ns = slice(c * nn, (c + 1) * nn)
        qr = q[ns].reshape(nn, L, H, D)  # [nl, h, qh, d]
        kr = k[ns].reshape(nn, L, H, D)
        QTh = np.empty((nn, 65, L, H), np.float32)
        QTh[:, :64] = qr.transpose(0, 3, 1, 2)  # [nl, d, h, qh]
        QTh[:, 64] = 1.0
        KTh = np.empty((nn, 65, L, H), np.float32)
        KTh[:, :64] = kr.transpose(0, 3, 1, 2)
        KTh[:, 64] = (m[ns].astype(np.float32) - 1.0)[:, :, None] * (-NEG)
        vperm = v[ns][:, PERM, :].reshape(nn, L, H, D)  # [nl, l, vh, d]
        VTh = np.ascontiguousarray(
            vperm.transpose(0, 1, 3, 2).reshape(nn, L, E)
        ).astype(ml_dtypes.bfloat16)
        QKh = np.concatenate(
            [QTh.reshape(nn, 65, E), KTh.reshape(nn, 65, E)], axis=2
        ).astype(ml_dtypes.bfloat16)
        maps.append({"qkt": QKh, "vt": VTh, "wh8": WHh, "wl8": WLh, "bb": BBh})
    return maps


def assemble_output(core_outs, nn=NN, n_cores=NCORES):
    """core_outs[c] = [nn*64, E] with row nl*64+kh -> full (256, 64, E)."""
    n_total = nn * n_cores
    full = np.empty((H, n_total, E), np.float32)  # [kh, n]
    for c in range(n_cores):
        full[:, c * nn : (c + 1) * nn, :] = (
            np.asarray(core_outs[c]).astype(np.float32)
            .reshape(nn, H, E)
            .transpose(1, 0, 2)
        )
    return full.reshape(n_total, L, E)


def kernel(**inputs) -> np.ndarray:
    from concourse import bass_utils

    key = (NN, NCORES)
    if key not in _PROGRAM_CACHE:
        _PROGRAM_CACHE[key] = build_program(NN, NCORES)
    nc = _PROGRAM_CACHE[key]

    in_maps = prep_inputs(inputs, NN, NCORES)
    trace = bool(int(os.environ.get("KERNEL_TRACE", "0")))
    res = bass_utils.run_bass_kernel_spmd(
        nc,
        in_maps,
        core_ids=list(range(NCORES)),
        trace=trace,
        trace_cores=list(range(NCORES)) if trace else None,
    )
    if trace and res.exec_time_ns is not None:
        print(f"HW exec time: {res.exec_time_ns} ns")
        print(f"HW exec time mean: {res.mean_exec_time_ns} ns")
    core_outs = [r["out"] for r in res.results]
    return assemble_output(core_outs, NN, NCORES)


# revision 42
# speedup vs baseline: 1.0404x; 1.0033x over previous
"""Trainium2 Bass kernel for nn_MultiHeadAttention_81149112090633.

Math (faithful to the quirky reference):
  energy[q,k,n,h] = sum_d query[n,h,q*64+d] * keys[n,h,k*64+d]
  energy masked with -inf where mask[n,h]==0, softmax over the BATCH axis n,
  out[q,k,n,d] = sum_h att[q,k,n,h] * vsum[n,h,d],  vsum = sum_vh values[n,h,vh*64+d]
  final = rows(k,n) x features(q,d) matrix,  Y = X @ w_out.T + b_out

Sharding: data-parallel over batch n (32 per core x 8 cores). The softmax
couples cores only through the per-(q,k,h) denominator S = sum_n exp(...);
S is combined with an on-device AllReduce (1 MB), everything else is local.

Per-core phases:
  P1: per n: 64 tiny matmuls -> energy psum [128,2048] (partition=(h%2)*64+q,
      col=(h//2)*64+k); masking folded into the matmul via an augmented 65th
      contraction row. ACT exp (scale=1/8) -> expm bf16; S += expm (split
      DVE/Pool, fp32); expm spilled to HBM. (V loads deferred to phase 2.)
  CC: AllReduce S; transposed readback -> S^T [h, (q,k)]; reciprocal -> RSb.
  P2: per n: transposed readback of expm -> att^T [h, (q,k)]; multiply by
      RSb; einsum2 matmuls put d on psum partitions par*64 (par=q%2) so the
      psum layout matches X^T directly; convert psum -> X^T as SPLIT fp8:
      hi = fp8(x) (ACT), lo = fp8(x - hi) (DVE). alpha=2 is folded into
      VSALL so psum already holds alpha*X.
  P3: Y = X @ W^T as fp8 DoubleRow matmuls (2 k-tiles of 128 per instr,
      0.5 cycles/row): per 256-wide output chunk accumulate 16 ci-pairs x
      3 terms (Xh*Wh + Xh*Wl + Xl*Wh); W is pre-split hi/lo fp8 on the host
      scaled by beta=64. Epilogue: ACT copy psum*(1/(alpha*beta)) -> bf16,
      Pool adds bias, DMA out (bf16; host converts to f32).
      P3 is interleaved with P2 in a staircase so the PE never starves; W is
      streamed twice total.

The h (seq) axis is stored partition-permuted (evens then odds) in phase 2;
values rows are pre-permuted on the host to match.
"""

import os

import numpy as np
import ml_dtypes

N, L, H, D, E = 256, 64, 64, 64, 4096
NCORES = 8
NN = N // NCORES  # 32 batch elements per core
NEG = -2000.0  # mask bias pre exp-scale (exp((e-2000)/8) == 0 in fp32)
ALPHA = 2.0  # X scale into fp8 (folded into VSALL)
BETA = 64.0  # W scale into fp8 (folded into host W prep)

# partition p in phase-2 h-layout corresponds to seq position PERM[p]
PERM = np.array([2 * p for p in range(32)] + [2 * p + 1 for p in range(32)])

_PROGRAM_CACHE = {}


def build_program(nn=NN, n_cores=NCORES, use_collective=True, max_phase=3):
    """Build the Bass/Tile SPMD program (one NeuronCore's instruction stream)."""
    import concourse.bass as bass
    import concourse.mybir as mybir
    import concourse.tile as tile
    from concourse import bacc

    f32 = mybir.dt.float32
    bf16 = mybir.dt.bfloat16
    f8 = mybir.dt.float8e4
    AF = mybir.ActivationFunctionType
    ALU = mybir.AluOpType
    DR = mybir.MatmulPerfMode.DoubleRow
    R = nn * 64  # output rows per core

    nc = bacc.Bacc(trn_type="TRN2", num_devices=n_cores)

    QKT = nc.dram_tensor("qkt", [nn, 65, 2 * E], bf16, kind="ExternalInput").ap()
    VT = nc.dram_tensor("vt", [nn, L, E], bf16, kind="ExternalInput").ap()
    WH8 = nc.dram_tensor("wh8", [128, 16 * 32 * 256], f8, kind="ExternalInput").ap()
    WL8 = nc.dram_tensor("wl8", [128, 16 * 32 * 256], f8, kind="ExternalInput").ap()
    BB = nc.dram_tensor("bb", [128, E], bf16, kind="ExternalInput").ap()
    OUT = nc.dram_tensor("out", [R, E], bf16, kind="ExternalOutput").ap()
    f16 = mybir.dt.float16
    EXPM = nc.dram_tensor("expmbuf", [nn, 128, 2048], bf16, kind="Internal").ap()
    CCIN = nc.dram_tensor("ccin", [128, 2048], f16, kind="Internal").ap()
    CCOUT = nc.dram_tensor(
        "ccout", [128, 2048], f16, kind="Internal", addr_space="Shared"
    ).ap()

    with tile.TileContext(nc) as tc:
        with tc.tile_pool(name="persist", bufs=1) as persist:
            VSALL = persist.tile([64, 64 * nn], bf16, tag="VSALL")
            RSb = persist.tile([64, E], bf16, tag="RSb")

            # ---------------- Phase 1 ----------------
            with (
                tc.tile_pool(name="p1", bufs=6) as p1,
                tc.tile_pool(name="p1q", bufs=5) as p1q,
                tc.tile_pool(name="p1s", bufs=1) as p1s,
                tc.tile_pool(name="ps1", bufs=2, space="PSUM") as psp,
            ):
                S = p1s.tile([128, 2048], f16, tag="S")
                for nl in range(nn):
                    qka = p1q.tile([65, 2 * E], bf16, tag="qka")
                    nc.sync.dma_start(qka[:], QKT[nl])
                    ps = psp.tile([128, 2048], f32, tag="ps")
                    for h in range(L):
                        par, j = h % 2, h // 2
                        nc.tensor.matmul(
                            ps[par * 64 : par * 64 + 64, j * 64 : j * 64 + 64],
                            qka[:, h * 64 : h * 64 + 64],
                            qka[:, E + h * 64 : E + h * 64 + 64],
                            start=True,
                            stop=True,
                        )
                    expm = p1.tile([128, 2048], bf16, tag="expm")
                    nc.scalar.activation(expm[:], ps[:], AF.Exp, scale=0.125)
                    with nc.allow_low_precision(reason="fp16 softmax denom, ~0.1% err"):
                        if nl == 0:
                            nc.vector.tensor_copy(S[:], expm[:])
                        else:
                            nc.vector.tensor_add(S[:], S[:], expm[:])
                    nc.scalar.dma_start(EXPM[nl], expm[:])

                # ---------------- AllReduce of S ----------------
                # Split into two q-halves pipelined so phase 2 can start on
                # the first half while the second is still in flight.
                ccr = CCOUT.rearrange("(par q) (j k) -> par j q k", par=2, k=64)
                ST = p1s.tile([64, E], f16, tag="ST")
                for qh in range(2):
                    r0 = qh * 32
                    nc.sync.dma_start(CCIN[r0 : r0 + 32], S[r0 : r0 + 32])
                    nc.sync.dma_start(CCIN[64 + r0 : 96 + r0], S[64 + r0 : 96 + r0])
                    if use_collective:
                        nc.gpsimd.collective_compute(
                            "AllReduce",
                            mybir.AluOpType.add,
                            replica_groups=[list(range(n_cores))],
                            ins=[CCIN[r0 : r0 + 32]],
                            outs=[CCOUT[r0 : r0 + 32]],
                        )
                        nc.gpsimd.collective_compute(
                            "AllReduce",
                            mybir.AluOpType.add,
                            replica_groups=[list(range(n_cores))],
                            ins=[CCIN[64 + r0 : 96 + r0]],
                            outs=[CCOUT[64 + r0 : 96 + r0]],
                        )
                    else:
                        nc.sync.dma_start(CCOUT[r0 : r0 + 32], CCIN[r0 : r0 + 32])
                        nc.sync.dma_start(
                            CCOUT[64 + r0 : 96 + r0], CCIN[64 + r0 : 96 + r0]
                        )
                    # transposed readback: S^T[p, q*64+k] with h=PERM[p]
                    nc.sync.dma_start(
                        ST[0:32, qh * 2048 : qh * 2048 + 2048], ccr[0][:, r0 : r0 + 32]
                    )
                    nc.sync.dma_start(
                        ST[32:64, qh * 2048 : qh * 2048 + 2048], ccr[1][:, r0 : r0 + 32]
                    )
                    with nc.allow_low_precision(reason="recip feeds bf16 att anyway"):
                        nc.vector.reciprocal(
                            RSb[:, qh * 2048 : qh * 2048 + 2048],
                            ST[:, qh * 2048 : qh * 2048 + 2048],
                        )

            # ---------------- Phase 2 + 3 ----------------
            with (
                tc.tile_pool(name="xt", bufs=1) as xtp,
                tc.tile_pool(name="p2", bufs=3) as p2,
                tc.tile_pool(name="pvt", bufs=1) as pvt,
                tc.tile_pool(name="p3w", bufs=2) as p3w,
                tc.tile_pool(name="p3b", bufs=2) as p3b,
                tc.tile_pool(name="p3y", bufs=2) as p3y,
                tc.tile_pool(name="ps2p", bufs=3, space="PSUM") as ps2p,
                tc.tile_pool(name="psyp", bufs=4, space="PSUM") as psyp,
            ):
                XTH = xtp.tile([128, 32 * R], f8, tag="XTH")
                XTL = xtp.tile([128, 32 * R], f8, tag="XTL")
                XTHv = XTH.rearrange("p (ci r) -> p ci r", r=R)
                XTLv = XTL.rearrange("p (ci r) -> p ci r", r=R)

                def load_vt(nl, q=None):
                    vt = pvt.tile([L, E], bf16, tag="vt")
                    (q or nc.scalar).dma_start(vt[:], VT[nl])
                    vsb = p2.tile([64, 64], bf16, tag="vsb")
                    vtv = vt.rearrange("p (d vh) -> p d vh", vh=64)
                    with nc.allow_low_precision(reason="vsum is stored bf16 anyway"):
                        for dq in range(4):
                            nc.vector.reduce_sum(
                                vsb[:, dq * 16 : dq * 16 + 16],
                                vtv[:, dq * 16 : dq * 16 + 16, :],
                                axis=mybir.AxisListType.X,
                            )
                    nc.scalar.activation(
                        VSALL[:, nl * 64 : nl * 64 + 64], vsb[:], AF.Copy, scale=ALPHA
                    )

                araws = {}

                def prefetch_araw(nl):
                    araw = p2.tile([64, E], bf16, tag="araw")
                    er = EXPM[nl].rearrange("(par q) (j k) -> par j q k", par=2, k=64)
                    # split by (parity, q-half): the q<32 halves land first so
                    # the first att-mul can start before the rest arrives
                    nc.sync.dma_start(araw[0:32, 0:2048], er[0][:, 0:32])
                    nc.sync.dma_start(araw[32:64, 0:2048], er[1][:, 0:32])
                    nc.sync.dma_start(araw[0:32, 2048:4096], er[0][:, 32:64])
                    nc.sync.dma_start(araw[32:64, 2048:4096], er[1][:, 32:64])
                    araws[nl] = araw

                wtiles = {}

                def load_w(oc):
                    wh = p3w.tile([128, 32 * 256], f8, tag="wh")
                    wl = p3w.tile([128, 32 * 256], f8, tag="wl")
                    nc.gpsimd.dma_start(wh[:], WH8[:, oc * 8192 : oc * 8192 + 8192])
                    nc.gpsimd.dma_start(wl[:], WL8[:, oc * 8192 : oc * 8192 + 8192])
                    bbt = p3b.tile([128, 256], bf16, tag="bbt")
                    nc.sync.dma_start(bbt[:], BB[:, oc * 256 : oc * 256 + 256])
                    wtiles[oc] = (wh, wl, bbt)

                def run_p3(oc, rc_list):
                    wh, wl, bbt = wtiles.pop(oc)
                    whv = wh.rearrange("p (ci o) -> p ci o", ci=32)
                    wlv = wl.rearrange("p (ci o) -> p ci o", ci=32)
                    for rc in rc_list:
                        psY = psyp.tile([128, 512], f32, tag="psy")
                        idx = 0
                        for j in range(16):
                            for xv, wv in ((XTHv, whv), (XTHv, wlv), (XTLv, whv)):
                                nc.tensor.matmul(
                                    psY[:, 0:256],
                                    xv[:, 2 * j : 2 * j + 2, rc * 128 : rc * 128 + 128],
                                    wv[:, 2 * j : 2 * j + 2, :],
                                    start=(idx == 0),
                                    stop=(idx == 47),
                                    perf_mode=DR,
                                )
                                idx += 1
                        t = p3y.tile([128, 256], bf16, tag="t")
                        nc.scalar.activation(
                            t[:], psY[:, 0:256], AF.Copy, scale=1.0 / (ALPHA * BETA)
                        )
                        yb = p3y.tile([128, 256], bf16, tag="yb")
                        nc.gpsimd.tensor_tensor(
                            out=yb[:], in0=t[:], in1=bbt[:], op=ALU.add
                        )
                        nc.gpsimd.dma_start(
                            OUT[rc * 128 : rc * 128 + 128, oc * 256 : oc * 256 + 256],
                            yb[:],
                        )

                # prefetches that overlap the S-collective chain
                prefetch_araw(0)
                prefetch_araw(1)
                load_vt(0, nc.sync)
                load_vt(1, nc.sync)
                load_w(0)

                for nl in range(nn if max_phase >= 2 else 0):
                    if max_phase >= 3 and nl % 2 == 0 and nl >= 2:
                        k = nl // 2
                        load_w(k)
                        run_p3(k - 1, list(range(0, k)))
                    if nl + 2 < nn:
                        prefetch_araw(nl + 2)
                        load_vt(nl + 2)
                    araw = araws.pop(nl)
                    nc.vector.tensor_mul(
                        araw[:, 0:2048], araw[:, 0:2048], RSb[:, 0:2048]
                    )
                    nc.vector.tensor_mul(
                        araw[:, 2048:4096], araw[:, 2048:4096], RSb[:, 2048:4096]
                    )
                    arv = araw.rearrange("p (q k) -> p q k", k=64)
                    for quarter in range(4):
                        ps2 = ps2p.tile([128, 512], f32, tag="ps2")
                        for par in range(2):
                            nc.tensor.matmul(
                                ps2[par * 64 : par * 64 + 64, :],
                                VSALL[:, nl * 64 : nl * 64 + 64],
                                arv[:, quarter * 16 + par : quarter * 16 + 16 : 2, :],
                                start=True,
                                stop=True,
                            )
                        ps2v = ps2.rearrange("p (t k) -> p t k", k=64)
                        dsth = XTHv[
                            :, quarter * 8 : quarter * 8 + 8, nl * 64 : nl * 64 + 64
                        ]
                        nc.scalar.copy(dsth, ps2v)
                        nc.vector.tensor_tensor(
                            out=XTLv[
                                :, quarter * 8 : quarter * 8 + 8, nl * 64 : nl * 64 + 64
                            ],
                            in0=ps2v,
                            in1=dsth,
                            op=ALU.subtract,
                        )
                # tail: oc 15 runs all rc, then each oc chunk is re-streamed
                # for its remaining rc range
                if max_phase >= 3:
                    run_p3(15, list(range(0, 16)))
                    load_w(0)
                    for oc in range(15):
                        if oc + 1 < 15:
                            load_w(oc + 1)
                        run_p3(oc, list(range(oc + 1, 16)))

    nc.compile()
    return nc


def prep_inputs(inputs, nn=NN, n_cores=NCORES):
    """Host-side shard + layout prep. Returns list of per-core input maps."""
    q = np.asarray(inputs["query"], dtype=np.float32)
    k = np.asarray(inputs["keys"], dtype=np.float32)
    v = np.asarray(inputs["values"], dtype=np.float32)
    m = np.asarray(inputs["mask"])
    w = np.asarray(inputs["w_out"], dtype=np.float32)
    b = np.asarray(inputs["b_out"], dtype=np.float32)

    f8 = ml_dtypes.float8_e4m3
    ws = np.ascontiguousarray(w.T) * BETA  # [i, o] = [ci*128+p, oc*256+o]
    WHh = ws.astype(f8)
    WLh = (ws - WHh.astype(np.float32)).astype(f8)
    # pre-tile to [p, oc, ci, o] so per-oc-chunk DMAs are fully contiguous
    WHh = np.ascontiguousarray(
        WHh.reshape(32, 128, 16, 256).transpose(1, 2, 0, 3).reshape(128, 16 * 32 * 256)
    )
    WLh = np.ascontiguousarray(
        WLh.reshape(32, 128, 16, 256).transpose(1, 2, 0, 3).reshape(128, 16 * 32 * 256)
    )
    BBh = np.ascontiguousarray(
        np.broadcast_to(b.astype(np.float32), (128, E))
    ).astype(ml_dtypes.bfloat16)

    maps = []
    for c in range(n_cores):
        ns = slice(c * nn, (c + 1) * nn)
        qr = q[ns].reshape(nn, L, H, D)  # [nl, h, qh, d]
        kr = k[ns].reshape(nn, L, H, D)
        QTh = np.empty((nn, 65, L, H), np.float32)
        QTh[:, :64] = qr.transpose(0, 3, 1, 2)  # [nl, d, h, qh]
        QTh[:, 64] = 1.0
        KTh = np.empty((nn, 65, L, H), np.float32)
        KTh[:, :64] = kr.transpose(0, 3, 1, 2)
        KTh[:, 64] = (m[ns].astype(np.float32) - 1.0)[:, :, None] * (-NEG)
        vperm = v[ns][:, PERM, :].reshape(nn, L, H, D)  # [nl, l, vh, d]
        VTh = np.ascontiguousarray(
            vperm.transpose(0, 1, 3, 2).reshape(nn, L, E)
        ).astype(ml_dtypes.bfloat16)
        QKh = np.concatenate(
            [QTh.reshape(nn, 65, E), KTh.reshape(nn, 65, E)], axis=2
        ).astype(ml_dtypes.bfloat16)
        maps.append({"qkt": QKh, "vt": VTh, "wh8": WHh, "wl8": WLh, "bb": BBh})
    return maps


def assemble_output(core_outs, nn=NN, n_cores=NCORES):
    """core_outs[c] = [nn*64, E] with row nl*64+kh -> full (256, 64, E)."""
    n_total = nn * n_cores
    full = np.empty((H, n_total, E), np.float32)  # [kh, n]
    for c in range(n_cores):
        full[:, c * nn : (c + 1) * nn, :] = (
            np.asarray(core_outs[c]).astype(np.float32)
            .reshape(nn, H, E)
            .transpose(1, 0, 2)
        )
    return full.reshape(n_total, L, E)


def kernel(**inputs) -> np.ndarray:
    from concourse import bass_utils

    key = (NN, NCORES)
    if key not in _PROGRAM_CACHE:
        _PROGRAM_CACHE[key] = build_program(NN, NCORES)
    nc = _PROGRAM_CACHE[key]

    in_maps = prep_inputs(inputs, NN, NCORES)
    trace = bool(int(os.environ.get("KERNEL_TRACE", "0")))
    res = bass_utils.run_bass_kernel_spmd(
        nc,
        in_maps,
        core_ids=list(range(NCORES)),
        trace=trace,
        trace_cores=list(range(NCORES)) if trace else None,
    )
    if trace and res.exec_time_ns is not None:
        print(f"HW exec time: {res.exec_time_ns} ns")
        print(f"HW exec time mean: {res.mean_exec_time_ns} ns")
    core_outs = [r["out"] for r in res.results]
    return assemble_output(core_outs, NN, NCORES)


# revision 47
# speedup vs baseline: 1.0477x; 1.0069x over previous
"""Trainium2 Bass kernel for nn_MultiHeadAttention_81149112090633.

Math (faithful to the quirky reference):
  energy[q,k,n,h] = sum_d query[n,h,q*64+d] * keys[n,h,k*64+d]
  energy masked with -inf where mask[n,h]==0, softmax over the BATCH axis n,
  out[q,k,n,d] = sum_h att[q,k,n,h] * vsum[n,h,d],  vsum = sum_vh values[n,h,vh*64+d]
  final = rows(k,n) x features(q,d) matrix,  Y = X @ w_out.T + b_out

Sharding: data-parallel over batch n (32 per core x 8 cores). The softmax
couples cores only through the per-(q,k,h) denominator S = sum_n exp(...);
S is combined with an on-device AllReduce (1 MB), everything else is local.

Per-core phases:
  P1: per n: 64 tiny matmuls -> energy psum [128,2048] (partition=(h%2)*64+q,
      col=(h//2)*64+k); masking folded into the matmul via an augmented 65th
      contraction row. ACT exp (scale=1/8) -> expm bf16; S += expm (split
      DVE/Pool, fp32); expm spilled to HBM. (V loads deferred to phase 2.)
  CC: AllReduce S; transposed readback -> S^T [h, (q,k)]; reciprocal -> RSb.
  P2: per n: transposed readback of expm -> att^T [h, (q,k)]; multiply by
      RSb; einsum2 matmuls put d on psum partitions par*64 (par=q%2) so the
      psum layout matches X^T directly; convert psum -> X^T as SPLIT fp8:
      hi = fp8(x) (ACT), lo = fp8(x - hi) (DVE). alpha=2 is folded into
      VSALL so psum already holds alpha*X.
  P3: Y = X @ W^T as fp8 DoubleRow matmuls (2 k-tiles of 128 per instr,
      0.5 cycles/row): per 256-wide output chunk accumulate 16 ci-pairs x
      3 terms (Xh*Wh + Xh*Wl + Xl*Wh); W is pre-split hi/lo fp8 on the host
      scaled by beta=64. Epilogue: ACT copy psum*(1/(alpha*beta)) -> bf16,
      Pool adds bias, DMA out (bf16; host converts to f32).
      P3 is interleaved with P2 in a staircase so the PE never starves; W is
      streamed twice total.

The h (seq) axis is stored partition-permuted (evens then odds) in phase 2;
values rows are pre-permuted on the host to match.
"""

import os

import numpy as np
import ml_dtypes

N, L, H, D, E = 256, 64, 64, 64, 4096
NCORES = 8
NN = N // NCORES  # 32 batch elements per core
NEG = -2000.0  # mask bias pre exp-scale (exp((e-2000)/8) == 0 in fp32)
ALPHA = 2.0  # X scale into fp8 (folded into VSALL)
BETA = 64.0  # W scale into fp8 (folded into host W prep)

# partition p in phase-2 h-layout corresponds to seq position PERM[p]
PERM = np.array([2 * p for p in range(32)] + [2 * p + 1 for p in range(32)])

_PROGRAM_CACHE = {}


def build_program(nn=NN, n_cores=NCORES, use_collective=True, max_phase=3):
    """Build the Bass/Tile SPMD program (one NeuronCore's instruction stream)."""
    import concourse.bass as bass
    import concourse.mybir as mybir
    import concourse.tile as tile
    from concourse import bacc

    f32 = mybir.dt.float32
    bf16 = mybir.dt.bfloat16
    f8 = mybir.dt.float8e4
    AF = mybir.ActivationFunctionType
    ALU = mybir.AluOpType
    DR = mybir.MatmulPerfMode.DoubleRow
    R = nn * 64  # output rows per core

    nc = bacc.Bacc(trn_type="TRN2", num_devices=n_cores)

    QKT = nc.dram_tensor("qkt", [nn, 65, 2 * E], bf16, kind="ExternalInput").ap()
    VT = nc.dram_tensor("vt", [nn, L, E], bf16, kind="ExternalInput").ap()
    WH8 = nc.dram_tensor("wh8", [128, 16 * 32 * 256], f8, kind="ExternalInput").ap()
    WL8 = nc.dram_tensor("wl8", [128, 16 * 32 * 256], f8, kind="ExternalInput").ap()
    BB = nc.dram_tensor("bb", [128, E], bf16, kind="ExternalInput").ap()
    OUT = nc.dram_tensor("out", [R, E], bf16, kind="ExternalOutput").ap()
    f16 = mybir.dt.float16
    EXPM = nc.dram_tensor("expmbuf", [nn, 128, 2048], bf16, kind="Internal").ap()
    CCIN = nc.dram_tensor("ccin", [128, 2048], f16, kind="Internal").ap()
    CCOUT = nc.dram_tensor(
        "ccout", [128, 2048], f16, kind="Internal", addr_space="Shared"
    ).ap()

    with tile.TileContext(nc) as tc:
        with (
            tc.tile_pool(name="persist", bufs=1) as persist,
            tc.tile_pool(name="p2", bufs=3) as p2,
            tc.tile_pool(name="pvt", bufs=1) as pvt,
        ):
            VSALL = persist.tile([64, 64 * nn], bf16, tag="VSALL")
            RSb = persist.tile([64, E], bf16, tag="RSb")

            def load_vt(nl, q=None):
                vt = pvt.tile([L, E], bf16, tag="vt")
                (q or nc.scalar).dma_start(vt[:], VT[nl])
                vsb = p2.tile([64, 64], bf16, tag="vsb")
                vtv = vt.rearrange("p (d vh) -> p d vh", vh=64)
                with nc.allow_low_precision(reason="vsum is stored bf16 anyway"):
                    for dq in range(4):
                        nc.vector.reduce_sum(
                            vsb[:, dq * 16 : dq * 16 + 16],
                            vtv[:, dq * 16 : dq * 16 + 16, :],
                            axis=mybir.AxisListType.X,
                        )
                nc.scalar.activation(
                    VSALL[:, nl * 64 : nl * 64 + 64], vsb[:], AF.Copy, scale=ALPHA
                )

            araws = {}

            def prefetch_araw(nl):
                araw = p2.tile([64, E], bf16, tag="araw")
                er = EXPM[nl].rearrange("(par q) (j k) -> par j q k", par=2, k=64)
                nc.sync.dma_start(araw[0:32, 0:2048], er[0][:, 0:32])
                nc.sync.dma_start(araw[32:64, 0:2048], er[1][:, 0:32])
                nc.sync.dma_start(araw[0:32, 2048:4096], er[0][:, 32:64])
                nc.sync.dma_start(araw[32:64, 2048:4096], er[1][:, 32:64])
                araws[nl] = araw

            # ---------------- Phase 1 ----------------
            with (
                tc.tile_pool(name="p1", bufs=6) as p1,
                tc.tile_pool(name="p1q", bufs=5) as p1q,
                tc.tile_pool(name="p1s", bufs=1) as p1s,
                tc.tile_pool(name="ps1", bufs=3, space="PSUM") as psp,
            ):
                S = p1s.tile([128, 2048], f16, tag="S")
                for nl in range(nn):
                    qka = p1q.tile([65, 2 * E], bf16, tag="qka")
                    nc.sync.dma_start(qka[:], QKT[nl])
                    expm = p1.tile([128, 2048], bf16, tag="expm")
                    # two psum half-tiles per n: exp/S-add of half A overlap
                    # the matmuls of half B, shortening the phase-1 tail
                    for half in range(2):
                        ph = psp.tile([128, 1024], f32, tag="ps")
                        for hh in range(32):
                            h = half * 32 + hh
                            par, j = h % 2, h // 2 - half * 16
                            nc.tensor.matmul(
                                ph[par * 64 : par * 64 + 64, j * 64 : j * 64 + 64],
                                qka[:, h * 64 : h * 64 + 64],
                                qka[:, E + h * 64 : E + h * 64 + 64],
                                start=True,
                                stop=True,
                            )
                        sl = slice(half * 1024, half * 1024 + 1024)
                        nc.scalar.activation(expm[:, sl], ph[:], AF.Exp, scale=0.125)
                        with nc.allow_low_precision(reason="fp16 softmax denom"):
                            if nl == 0:
                                nc.vector.tensor_copy(S[:, sl], expm[:, sl])
                            else:
                                nc.vector.tensor_add(S[:, sl], S[:, sl], expm[:, sl])
                    nc.scalar.dma_start(EXPM[nl], expm[:])

                # ---------------- AllReduce of S ----------------
                # Split into two q-halves pipelined so phase 2 can start on
                # the first half while the second is still in flight.
                ccr = CCOUT.rearrange("(par q) (j k) -> par j q k", par=2, k=64)
                ST = p1s.tile([64, E], f16, tag="ST")
                for qh in range(2):
                    r0 = qh * 32
                    nc.sync.dma_start(CCIN[r0 : r0 + 32], S[r0 : r0 + 32])
                    nc.sync.dma_start(CCIN[64 + r0 : 96 + r0], S[64 + r0 : 96 + r0])
                    if use_collective:
                        nc.gpsimd.collective_compute(
                            "AllReduce",
                            mybir.AluOpType.add,
                            replica_groups=[list(range(n_cores))],
                            ins=[CCIN[r0 : r0 + 32]],
                            outs=[CCOUT[r0 : r0 + 32]],
                        )
                        nc.gpsimd.collective_compute(
                            "AllReduce",
                            mybir.AluOpType.add,
                            replica_groups=[list(range(n_cores))],
                            ins=[CCIN[64 + r0 : 96 + r0]],
                            outs=[CCOUT[64 + r0 : 96 + r0]],
                        )
                    else:
                        nc.sync.dma_start(CCOUT[r0 : r0 + 32], CCIN[r0 : r0 + 32])
                        nc.sync.dma_start(
                            CCOUT[64 + r0 : 96 + r0], CCIN[64 + r0 : 96 + r0]
                        )
                    # transposed readback: S^T[p, q*64+k] with h=PERM[p]
                    nc.sync.dma_start(
                        ST[0:32, qh * 2048 : qh * 2048 + 2048], ccr[0][:, r0 : r0 + 32]
                    )
                    nc.sync.dma_start(
                        ST[32:64, qh * 2048 : qh * 2048 + 2048], ccr[1][:, r0 : r0 + 32]
                    )
                    with nc.allow_low_precision(reason="recip feeds bf16 att anyway"):
                        nc.vector.reciprocal(
                            RSb[:, qh * 2048 : qh * 2048 + 2048],
                            ST[:, qh * 2048 : qh * 2048 + 2048],
                        )
                    prefetch_araw(qh)
                    load_vt(qh, nc.sync)

            # ---------------- Phase 2 + 3 ----------------
            with (
                tc.tile_pool(name="xt", bufs=1) as xtp,
                tc.tile_pool(name="p3w", bufs=2) as p3w,
                tc.tile_pool(name="p3b", bufs=2) as p3b,
                tc.tile_pool(name="p3y", bufs=2) as p3y,
                tc.tile_pool(name="ps2p", bufs=3, space="PSUM") as ps2p,
                tc.tile_pool(name="psyp", bufs=5, space="PSUM") as psyp,
            ):
                XTH = xtp.tile([128, 32 * R], f8, tag="XTH")
                XTL = xtp.tile([128, 32 * R], f8, tag="XTL")
                XTHv = XTH.rearrange("p (ci r) -> p ci r", r=R)
                XTLv = XTL.rearrange("p (ci r) -> p ci r", r=R)

                wtiles = {}

                def load_w(oc):
                    wh = p3w.tile([128, 32 * 256], f8, tag="wh")
                    wl = p3w.tile([128, 32 * 256], f8, tag="wl")
                    nc.gpsimd.dma_start(wh[:], WH8[:, oc * 8192 : oc * 8192 + 8192])
                    nc.gpsimd.dma_start(wl[:], WL8[:, oc * 8192 : oc * 8192 + 8192])
                    bbt = p3b.tile([128, 256], bf16, tag="bbt")
                    nc.sync.dma_start(bbt[:], BB[:, oc * 256 : oc * 256 + 256])
                    wtiles[oc] = (wh, wl, bbt)

                def run_p3(oc, rc_list):
                    wh, wl, bbt = wtiles.pop(oc)
                    whv = wh.rearrange("p (ci o) -> p ci o", ci=32)
                    wlv = wl.rearrange("p (ci o) -> p ci o", ci=32)
                    for rc in rc_list:
                        psY = psyp.tile([128, 512], f32, tag="psy")
                        idx = 0
                        for j in range(16):
                            for xv, wv in ((XTHv, whv), (XTHv, wlv), (XTLv, whv)):
                                nc.tensor.matmul(
                                    psY[:, 0:256],
                                    xv[:, 2 * j : 2 * j + 2, rc * 128 : rc * 128 + 128],
                                    wv[:, 2 * j : 2 * j + 2, :],
                                    start=(idx == 0),
                                    stop=(idx == 47),
                                    perf_mode=DR,
                                )
                                idx += 1
                        t = p3y.tile([128, 256], bf16, tag="t")
                        nc.scalar.activation(
                            t[:], psY[:, 0:256], AF.Copy, scale=1.0 / (ALPHA * BETA)
                        )
                        yb = p3y.tile([128, 256], bf16, tag="yb")
                        nc.gpsimd.tensor_tensor(
                            out=yb[:], in0=t[:], in1=bbt[:], op=ALU.add
                        )
                        nc.gpsimd.dma_start(
                            OUT[rc * 128 : rc * 128 + 128, oc * 256 : oc * 256 + 256],
                            yb[:],
                        )

                # prefetches that overlap the S-collective chain
                load_w(0)

                for nl in range(nn if max_phase >= 2 else 0):
                    if max_phase >= 3 and nl % 2 == 0 and nl >= 2:
                        k = nl // 2
                        load_w(k)
                        run_p3(k - 1, list(range(0, k)))
                    if nl + 2 < nn:
                        prefetch_araw(nl + 2)
                        load_vt(nl + 2)
                    araw = araws.pop(nl)
                    nc.vector.tensor_mul(
                        araw[:, 0:2048], araw[:, 0:2048], RSb[:, 0:2048]
                    )
                    nc.vector.tensor_mul(
                        araw[:, 2048:4096], araw[:, 2048:4096], RSb[:, 2048:4096]
                    )
                    arv = araw.rearrange("p (q k) -> p q k", k=64)
                    for quarter in range(4):
                        ps2 = ps2p.tile([128, 512], f32, tag="ps2")
                        for par in range(2):
                            nc.tensor.matmul(
                                ps2[par * 64 : par * 64 + 64, :],
                                VSALL[:, nl * 64 : nl * 64 + 64],
                                arv[:, quarter * 16 + par : quarter * 16 + 16 : 2, :],
                                start=True,
                                stop=True,
                            )
                        ps2v = ps2.rearrange("p (t k) -> p t k", k=64)
                        dsth = XTHv[
                            :, quarter * 8 : quarter * 8 + 8, nl * 64 : nl * 64 + 64
                        ]
                        nc.scalar.copy(dsth, ps2v)
                        nc.vector.tensor_tensor(
                            out=XTLv[
                                :, quarter * 8 : quarter * 8 + 8, nl * 64 : nl * 64 + 64
                            ],
                            in0=ps2v,
                            in1=dsth,
                            op=ALU.subtract,
                        )
                # tail: oc 15 runs all rc, then each oc chunk is re-streamed
                # for its remaining rc range
                if max_phase >= 3:
                    run_p3(15, list(range(0, 16)))
                    load_w(0)
                    for oc in range(15):
                        if oc + 1 < 15:
                            load_w(oc + 1)
                        run_p3(oc, list(range(oc + 1, 16)))

    nc.compile()
    return nc


def prep_inputs(inputs, nn=NN, n_cores=NCORES):
    """Host-side shard + layout prep. Returns list of per-core input maps."""
    q = np.asarray(inputs["query"], dtype=np.float32)
    k = np.asarray(inputs["keys"], dtype=np.float32)
    v = np.asarray(inputs["values"], dtype=np.float32)
    m = np.asarray(inputs["mask"])
    w = np.asarray(inputs["w_out"], dtype=np.float32)
    b = np.asarray(inputs["b_out"], dtype=np.float32)

    f8 = ml_dtypes.float8_e4m3
    ws = np.ascontiguousarray(w.T) * BETA  # [i, o] = [ci*128+p, oc*256+o]
    WHh = ws.astype(f8)
    WLh = (ws - WHh.astype(np.float32)).astype(f8)
    # pre-tile to [p, oc, ci, o] so per-oc-chunk DMAs are fully contiguous
    WHh = np.ascontiguousarray(
        WHh.reshape(32, 128, 16, 256).transpose(1, 2, 0, 3).reshape(128, 16 * 32 * 256)
    )
    WLh = np.ascontiguousarray(
        WLh.reshape(32, 128, 16, 256).transpose(1, 2, 0, 3).reshape(128, 16 * 32 * 256)
    )
    BBh = np.ascontiguousarray(
        np.broadcast_to(b.astype(np.float32), (128, E))
    ).astype(ml_dtypes.bfloat16)

    maps = []
    for c in range(n_cores):
        ns = slice(c * nn, (c + 1) * nn)
        qr = q[ns].reshape(nn, L, H, D)  # [nl, h, qh, d]
        kr = k[ns].reshape(nn, L, H, D)
        QTh = np.empty((nn, 65, L, H), np.float32)
        QTh[:, :64] = qr.transpose(0, 3, 1, 2)  # [nl, d, h, qh]
        QTh[:, 64] = 1.0
        KTh = np.empty((nn, 65, L, H), np.float32)
        KTh[:, :64] = kr.transpose(0, 3, 1, 2)
        KTh[:, 64] = (m[ns].astype(np.float32) - 1.0)[:, :, None] * (-NEG)
        vperm = v[ns][:, PERM, :].reshape(nn, L, H, D)  # [nl, l, vh, d]
        VTh = np.ascontiguousarray(
            vperm.transpose(0, 1, 3, 2).reshape(nn, L, E)
        ).astype(ml_dtypes.bfloat16)
        QKh = np.concatenate(
            [QTh.reshape(nn, 65, E), KTh.reshape(nn, 65, E)], axis=2
        ).astype(ml_dtypes.bfloat16)
        maps.append({"qkt": QKh, "vt": VTh, "wh8": WHh, "wl8": WLh, "bb": BBh})
    return maps


def assemble_output(core_outs, nn=NN, n_cores=NCORES):
    """core_outs[c] = [nn*64, E] with row nl*64+kh -> full (256, 64, E)."""
    n_total = nn * n_cores
    full = np.empty((H, n_total, E), np.float32)  # [kh, n]
    for c in range(n_cores):
        full[:, c * nn : (c + 1) * nn, :] = (
            np.asarray(core_outs[c]).astype(np.float32)
            .reshape(nn, H, E)
            .transpose(1, 0, 2)
        )
    return full.reshape(n_total, L, E)


def kernel(**inputs) -> np.ndarray:
    from concourse import bass_utils

    key = (NN, NCORES)
    if key not in _PROGRAM_CACHE:
        _PROGRAM_CACHE[key] = build_program(NN, NCORES)
    nc = _PROGRAM_CACHE[key]

    in_maps = prep_inputs(inputs, NN, NCORES)
    trace = bool(int(os.environ.get("KERNEL_TRACE", "0")))
    res = bass_utils.run_bass_kernel_spmd(
        nc,
        in_maps,
        core_ids=list(range(NCORES)),
        trace=trace,
        trace_cores=list(range(NCORES)) if trace else None,
    )
    if trace and res.exec_time_ns is not None:
        print(f"HW exec time: {res.exec_time_ns} ns")
        print(f"HW exec time mean: {res.mean_exec_time_ns} ns")
    core_outs = [r["out"] for r in res.results]
    return assemble_output(core_outs, NN, NCORES)
